# revision 13
# baseline (speedup 1.0000x reference)
"""Trainium2 Bass kernel for sparse CausalSelfAttention (8 full heads W=1024,
8 reduced-qk heads W=256), SPMD over 8 NeuronCores.

Sharding: core c -> batch c//4, head-group g=c%4 (full heads 2g,2g+1 and
reduced heads 2g,2g+1). Q=128 attention tiling for both head groups (window
1024 = 8 tiles aligns so interior score tiles are mask-free; the two edge
tiles use complementary 128x128 triangle masks). Projections and c_proj are
woven between attention score groups so PE never waits on the exp chain.
All operands bf16 (psum accumulation f32). Host sums the 4 partial c_proj
outputs per batch element.
"""

import numpy as np

import concourse.bacc as bacc
import concourse.mybir as mybir
from concourse import bass_utils
from concourse.tile import TileContext

# problem constants (hardcoded; kernel.py must be self-contained)
B, T, C = 2, 2048, 1024
HDIM = 64          # full head dim (and v dim of reduced heads)
RDIM = 32          # reduced qk dim
NTT = T // 128     # 16 query tiles of 128
NREG = T // 512    # 4 regions of 4 tiles
N_CORES = 8
NK = C // 128      # k-tiles over C contraction

F32 = mybir.dt.float32
BF16 = mybir.dt.bfloat16

FULL_SPAN = 8      # window 1024 / 128
RED_SPAN = 2       # window 256 / 128


def _make_masks(nc, m_sb):
    """m_sb[:, 0] = lower-strict triangle (f < p); m_sb[:, 1] = upper (f >= p)."""
    nc.gpsimd.memset(m_sb[:], 1.0)
    # m_lo: keep where (-1 - f + p) >= 0
    nc.gpsimd.affine_select(out=m_sb[:, 0, :], in_=m_sb[:, 0, :],
                            compare_op=mybir.AluOpType.is_ge, fill=0.0,
                            base=-1, pattern=[[-1, 128]], channel_multiplier=1)
    # m_hi: keep where (f - p) >= 0
    nc.gpsimd.affine_select(out=m_sb[:, 1, :], in_=m_sb[:, 1, :],
                            compare_op=mybir.AluOpType.is_ge, fill=0.0,
                            base=0, pattern=[[1, 128]], channel_multiplier=-1)


class Emitter:
    def __init__(self, nc, pools, aps):
        self.nc = nc
        (self.wpool, self.xpool, self.xbpool, self.qkpool, self.ppool,
         self.opool, self.rpool, self.ps_misc, self.ps_s, self.ps_y) = pools
        (self.xT, self.wq, self.wk, self.wqkr, self.wv, self.wproj,
         self.out) = aps

    def setup_tiles(self):
        nc = self.nc
        w = self.wpool
        self.wq_sb = w.tile([128, NK, 128], BF16, tag="wq")
        self.wk_sb = w.tile([128, NK, 128], BF16, tag="wk")
        self.wqkr_sb = w.tile([128, NK, 128], BF16, tag="wqkr")
        self.wv_sb = w.tile([128, NK, 256], BF16, tag="wv")
        self.wproj_sb = w.tile([128, 2, C], BF16, tag="wproj")
        self.m_sb = w.tile([128, 2, 128], BF16, tag="m")
        _make_masks(nc, self.m_sb)

        qk = self.qkpool
        # q tiles are zero-padded per head so every score matmul contracts the
        # full partition range at tile_position (0,0): bf16 matmuls crash when
        # consecutive instructions change partition offset (HW-probed)
        self.qTfA = qk.tile([128, T], BF16, tag="qTfA")
        self.qTfB = qk.tile([128, T], BF16, tag="qTfB")
        nc.gpsimd.memset(self.qTfA[64:128, :], 0.0)
        nc.gpsimd.memset(self.qTfB[0:64, :], 0.0)
        self.kTf = qk.tile([128, T], BF16, tag="kTf")
        self.qTrA = qk.tile([64, T], BF16, tag="qTrA")
        self.qTrB = qk.tile([64, T], BF16, tag="qTrB")
        nc.gpsimd.memset(self.qTrA[32:64, :], 0.0)
        nc.gpsimd.memset(self.qTrB[0:32, :], 0.0)
        self.kTr = qk.tile([64, T], BF16, tag="kTr")
        self.v_sb = qk.tile([128, NTT, 4, 128], BF16, tag="v")
        nc.gpsimd.memset(self.v_sb[:, :, :, 64:128], 1.0)
        self.yTf = qk.tile([128, T], BF16, tag="yTf")
        self.yTr = qk.tile([128, T], BF16, tag="yTr")

    # ---- projections ----------------------------------------------------
    def prologue_dma(self):
        """All input DMAs, ordered by first use. x block 0 arrives in four
        128-column chunks so the first projection slabs start early."""
        nc = self.nc
        xT3 = self.xT.rearrange("(k p) t -> p k t", p=128)
        nc.sync.dma_start(self.wq_sb[:], self.wq)
        self.xb0 = self.xbpool.tile([128, NK, 512], BF16, tag="xb0")
        nc.sync.dma_start(self.xb0[:, 0:4, 0:256], xT3[:, 0:4, 0:256])
        nc.sync.dma_start(self.xb0[:, 4:NK, 0:256], xT3[:, 4:NK, 0:256])
        nc.sync.dma_start(self.wk_sb[:], self.wk)
        nc.sync.dma_start(self.wqkr_sb[:], self.wqkr)
        nc.sync.dma_start(self.xb0[:, :, 256:512], xT3[:, :, 256:512])
        nc.sync.dma_start(self.wv_sb[:], self.wv)
        self.xbs = [None]
        for f in range(1, NREG):
            xb = self.xbpool.tile([128, NK, 512], BF16, tag="xb")
            self.xbs.append(xb)

        def xb_dma(f):
            nc.sync.dma_start(self.xbs[f][:], xT3[:, :, f * 512:(f + 1) * 512])
        xb_dma(1)
        nc.sync.dma_start(self.wproj_sb[:], self.wproj)
        xb_dma(2)
        xb_dma(3)

    def proj0_chunk_units(self, c):
        """Projection units for 256-column chunk c (0 or 1) of x block 0."""
        nc = self.nc
        csl = slice(c * 256, (c + 1) * 256)
        xk = [self.xb0[:, k, csl] for k in range(NK)]
        units = []

        def slab(w_sb, do_copy):
            psq = [None]

            def mk(k0):
                def u():
                    if k0 == 0:
                        psq[0] = self.ps_misc.tile([128, 256], F32, tag="m",
                                                   name="psc")
                    for k in range(k0, k0 + 4):
                        nc.tensor.matmul(psq[0][:], w_sb[:, k, :], xk[k],
                                         start=(k == 0), stop=(k == NK - 1))
                    if k0 == 4:
                        do_copy(psq[0])
                return u
            return [mk(0), mk(4)]

        def q_copy(ps):
            nc.scalar.copy(self.qTfA[0:64, csl], ps[0:64, :])
            nc.scalar.copy(self.qTfB[64:128, csl], ps[64:128, :])
        units += slab(self.wq_sb, q_copy)
        units += slab(self.wk_sb,
                      lambda ps: nc.scalar.copy(self.kTf[:, csl], ps[:]))

        def qkr_copy(ps):
            nc.vector.tensor_copy(self.qTrA[0:32, csl], ps[0:32, :])
            nc.vector.tensor_copy(self.qTrB[32:64, csl], ps[32:64, :])
            nc.vector.tensor_copy(self.kTr[:, csl], ps[64:128, :])
        units += slab(self.wqkr_sb, qkr_copy)

        for tt in (2 * c, 2 * c + 1):
            psv = [None]

            def mkv(tt, k0):
                def u():
                    if k0 == 0:
                        psv[0] = self.ps_misc.tile([128, 256], F32, tag="m",
                                                   name="psv")
                    for k in range(k0, k0 + 4):
                        nc.tensor.matmul(
                            psv[0][:],
                            xk[k][:, (tt % 2) * 128:(tt % 2) * 128 + 128],
                            self.wv_sb[:, k, :],
                            start=(k == 0), stop=(k == NK - 1))
                    if k0 == 4:
                        eng = (nc.vector.tensor_copy if tt % 2
                               else nc.scalar.copy)
                        eng(self.v_sb[:, tt, :, 0:64],
                            psv[0][:].rearrange("p (h d) -> p h d", h=4))
                return u
            units += [mkv(tt, 0), mkv(tt, 4)]
        return units

    def proj_units(self, f, xts=None):
        """Filler units projecting x block f into qTf/kTf/qTr/kTr/v_sb."""
        nc = self.nc
        if xts is None:
            xb = self.xbs[f]
            xts = [xb[:, k, :] for k in range(NK)]
        sl = slice(f * 512, (f + 1) * 512)
        units = []

        def slab(w_sb, do_copy):
            psq = [None]

            def mk(k0):
                def u():
                    if k0 == 0:
                        psq[0] = self.ps_misc.tile([128, 512], F32, tag="m", name="psq")
                    for k in (k0, k0 + 1):
                        nc.tensor.matmul(psq[0][:], w_sb[:, k, :], xts[k],
                                         start=(k == 0), stop=(k == NK - 1))
                    if k0 == NK - 2:
                        do_copy(psq[0])
                return u
            return [mk(k0) for k0 in range(0, NK, 2)]

        def q_copy(ps):
            nc.scalar.copy(self.qTfA[0:64, sl], ps[0:64, :])
            nc.scalar.copy(self.qTfB[64:128, sl], ps[64:128, :])
        units += slab(self.wq_sb, q_copy)
        units += slab(self.wk_sb,
                      lambda ps: nc.scalar.copy(self.kTf[:, sl], ps[:]))

        def qkr_copy(ps):
            nc.vector.tensor_copy(self.qTrA[0:32, sl], ps[0:32, :])
            nc.vector.tensor_copy(self.qTrB[32:64, sl], ps[32:64, :])
            nc.vector.tensor_copy(self.kTr[:, sl], ps[64:128, :])
        units += slab(self.wqkr_sb, qkr_copy)

        for tt in range(4 * f, 4 * f + 4):
            psv = [None]

            def mkv(tt, k0):
                def u():
                    if k0 == 0:
                        psv[0] = self.ps_misc.tile([128, 256], F32, tag="m", name="psv")
                    for k in range(k0, k0 + 4):
                        nc.tensor.matmul(
                            psv[0][:],
                            xts[k][:, (tt % 4) * 128:(tt % 4) * 128 + 128],
                            self.wv_sb[:, k, :],
                            start=(k == 0), stop=(k == NK - 1))
                    if k0 == 4:
                        eng = nc.vector.tensor_copy if tt % 2 else nc.scalar.copy
                        eng(self.v_sb[:, tt, :, 0:64],
                            psv[0][:].rearrange("p (h d) -> p h d", h=4))
                return u
            units += [mkv(tt, 0), mkv(tt, 4)]
        return units

    # ---- attention ------------------------------------------------------
    def attn_qb_units(self, qb, is_full):
        """Units for one 128-query block: score groups (shared exp) + PV
        accumulation + normalization."""
        nc = self.nc
        span = FULL_SPAN if is_full else RED_SPAN
        if is_full:
            qTs, kT, krows = (self.qTfA, self.qTfB), self.kTf, slice(0, 128)
        else:
            qTs, kT, krows = (self.qTrA, self.qTrB), self.kTr, slice(0, 64)
        yT = self.yTf if is_full else self.yTr
        vh0 = 0 if is_full else 2
        ks = list(range(max(0, qb - span), qb + 1))
        groups = []
        i = len(ks)
        while i > 0:
            groups.insert(0, ks[max(0, i - 4):i])
            i -= 4
        qsl = slice(qb * 128, (qb + 1) * 128)
        py = [None]
        shared = {}
        units = []

        def mk_scores(g, first):
            def u():
                if first:
                    # heads in separate psum banks (h stride = 2KB): each
                    # head's PV chain must be sole writer of its bank
                    py[0] = self.ps_y.tile([128, 2, 512], F32, tag="y",
                                           name="py")
                glen = len(g)
                pss = self.ps_s.tile([128, glen, 2, 128], F32, tag="s",
                                     name="pss")
                shared["pss"] = pss
                for i, kt in enumerate(g):
                    ksl = slice(kt * 128, (kt + 1) * 128)
                    for h in range(2):
                        # full contraction vs zero-padded q: keeps every
                        # matmul at tile_position (0,0); each strip written
                        # exactly once with its own start+stop
                        nc.tensor.matmul(
                            pss[:, i, h, :], kT[krows, ksl], qTs[h][krows, qsl],
                            start=True, stop=True)
                p_sb = self.ppool.tile([128, glen, 2, 128], BF16, tag="p",
                                       name="psb")
                shared["p_sb"] = p_sb
                nc.scalar.activation(p_sb[:], pss[:],
                                     mybir.ActivationFunctionType.Exp)
                for i, kt in enumerate(g):
                    midx = 0 if kt == qb - span else (1 if kt == qb else None)
                    if midx is not None:
                        mm = self.m_sb[:, midx, :].rearrange(
                            "p (a q) -> p a q", a=1).broadcast_to([128, 2, 128])
                        nc.vector.tensor_mul(p_sb[:, i, :, :],
                                             p_sb[:, i, :, :], mm)
            return u

        def mk_pv(g, first, last):
            def u():
                p_sb = shared["p_sb"]
                for i, kt in enumerate(g):
                    for h in range(2):
                        nc.tensor.matmul(py[0][:, h, 0:128],
                                         self.v_sb[:, kt, vh0 + h, :],
                                         p_sb[:, i, h, :],
                                         start=(first and i == 0),
                                         stop=(last and i == len(g) - 1))
                if last:
                    r_sb = self.rpool.tile([64, 2, 128], F32, tag="r")
                    nc.vector.reciprocal(r_sb[:], py[0][64:128, :, 0:128])
                    for h in range(2):
                        nc.vector.tensor_mul(yT[h * 64:(h + 1) * 64, qsl],
                                             py[0][0:64, h, 0:128],
                                             r_sb[:, h, :])
            return u

        for gi, g in enumerate(groups):
            units.append(mk_scores(g, gi == 0))
            units.append(mk_pv(g, gi == 0, gi == len(groups) - 1))
        return units

    # ---- c_proj ---------------------------------------------------------
    def cproj_units(self, f):
        nc = self.nc
        units = []
        for tt in range(4 * f, 4 * f + 4):
            tsl = slice(tt * 128, (tt + 1) * 128)
            o_sb = [None]

            def mk(tt, tsl, nb):
                def u():
                    if nb == 0:
                        o_sb[0] = self.opool.tile([128, C], BF16, tag="o", name="osb")
                    nsl = slice(nb * 512, (nb + 1) * 512)
                    pso = self.ps_misc.tile([128, 512], F32, tag="m")
                    nc.tensor.matmul(pso[:], self.yTf[:, tsl],
                                     self.wproj_sb[:, 0, nsl],
                                     start=True, stop=False)
                    nc.tensor.matmul(pso[:], self.yTr[:, tsl],
                                     self.wproj_sb[:, 1, nsl],
                                     start=False, stop=True)
                    if nb == 0:
                        nc.scalar.copy(o_sb[0][:, nsl], pso[:])
                    else:
                        nc.vector.tensor_copy(o_sb[0][:, nsl], pso[:])
                    nc.sync.dma_start(self.out[tsl, nsl], o_sb[0][:, nsl])
                return u
            units += [mk(tt, tsl, 0), mk(tt, tsl, 1)]
        return units

    # ---- weave ----------------------------------------------------------
    def region(self, f, fillers, inline_cproj=False):
        units = []
        cp = self.cproj_units(f) if inline_cproj else [None] * 8
        for i, qb in enumerate(range(4 * f, 4 * f + 4)):
            units += self.attn_qb_units(qb, True)
            units += self.attn_qb_units(qb, False)
            if inline_cproj:
                units += cp[2 * i:2 * i + 2]
        if inline_cproj:
            units += cp[2:4] and []  # cp consumed above per tt
        fi = 0
        for i, u in enumerate(units):
            u()
            want = (i + 1) * len(fillers) // len(units)
            while fi < want:
                fillers[fi]()
                fi += 1

    def emit(self):
        self.setup_tiles()
        self.prologue_dma()
        for u in self.proj0_chunk_units(0):
            u()
        self.region(0, self.proj0_chunk_units(1) + self.proj_units(1))
        self.region(1, self.proj_units(2) + self.cproj_units(0))
        cp1 = self.cproj_units(1)
        self.region(2, self.proj_units(3) + cp1[0:4])
        self.region(3, cp1[4:8] + self.cproj_units(2), inline_cproj=True)


def _build_nc(reps=1):
    nc = bacc.Bacc(trn_type="TRN2", target_bir_lowering=False, debug=False,
                   num_devices=1)

    xT = nc.dram_tensor("xT", [C, T], BF16, kind="ExternalInput").ap()
    # weights stored pre-arranged in the SBUF tile layout so each DMA row is
    # a single long contiguous run (sub-512B runs halve DMA bandwidth)
    wq = nc.dram_tensor("wq", [128, NK, 128], BF16, kind="ExternalInput").ap()
    wk = nc.dram_tensor("wk", [128, NK, 128], BF16, kind="ExternalInput").ap()
    wqkr = nc.dram_tensor("wqkr", [128, NK, 128], BF16,
                          kind="ExternalInput").ap()
    wv = nc.dram_tensor("wv", [128, NK, 256], BF16, kind="ExternalInput").ap()
    wproj = nc.dram_tensor("wproj", [128, 2, C], BF16,
                           kind="ExternalInput").ap()
    out = nc.dram_tensor("o", [T, C], BF16, kind="ExternalOutput").ap()
    aps = (xT, wq, wk, wqkr, wv, wproj, out)

    with TileContext(nc) as tc:
        with (
            tc.tile_pool(name="wpool", bufs=1) as wpool,
            tc.tile_pool(name="xpool", bufs=10) as xpool,
            tc.tile_pool(name="xbpool", bufs=4) as xbpool,
            tc.tile_pool(name="qk", bufs=1) as qkpool,
            tc.tile_pool(name="ppool", bufs=4) as ppool,
            tc.tile_pool(name="opool", bufs=3) as opool,
            tc.tile_pool(name="rpool", bufs=4) as rpool,
            tc.tile_pool(name="ps_misc", bufs=2, space="PSUM") as ps_misc,
            tc.tile_pool(name="ps_s", bufs=2, space="PSUM") as ps_s,
            tc.tile_pool(name="ps_y", bufs=1, space="PSUM") as ps_y,
        ):
            pools = (wpool, xpool, xbpool, qkpool, ppool, opool, rpool,
                     ps_misc, ps_s, ps_y)
            for _ in range(reps):
                Emitter(nc, pools, aps).emit()

    nc.compile()
    return nc


_NC_CACHE = {}


def _get_nc(reps=1):
    if reps not in _NC_CACHE:
        _NC_CACHE[reps] = _build_nc(reps)
    return _NC_CACHE[reps]


def make_in_maps(x, w_qkv_full, w_qk_red, w_v_red, w_proj):
    import ml_dtypes
    bf = ml_dtypes.bfloat16
    x = np.asarray(x, np.float32)
    w_qkv_full = np.asarray(w_qkv_full, np.float32)
    w_qk_red = np.asarray(w_qk_red, np.float32)
    w_v_red = np.asarray(w_v_red, np.float32)
    w_proj = np.asarray(w_proj, np.float32)
    sf = np.float32(1.0 / np.sqrt(HDIM))
    sr = np.float32(1.0 / np.sqrt(RDIM))
    in_maps = []
    for c in range(N_CORES):
        b, g = divmod(c, 4)
        hA, hB = 2 * g, 2 * g + 1
        wq = np.concatenate([w_qkv_full[:, 64 * hA:64 * hA + 64],
                             w_qkv_full[:, 64 * hB:64 * hB + 64]], 1) * sf
        wk = np.concatenate([w_qkv_full[:, 512 + 64 * hA:512 + 64 * hA + 64],
                             w_qkv_full[:, 512 + 64 * hB:512 + 64 * hB + 64]], 1)
        wqkr = np.concatenate(
            [w_qk_red[:, 32 * hA:32 * hA + 32] * sr,
             w_qk_red[:, 32 * hB:32 * hB + 32] * sr,
             w_qk_red[:, 256 + 32 * hA:256 + 32 * hA + 32],
             w_qk_red[:, 256 + 32 * hB:256 + 32 * hB + 32]], 1)
        wv = np.concatenate([w_qkv_full[:, 1024 + 64 * hA:1024 + 64 * hA + 64],
                             w_qkv_full[:, 1024 + 64 * hB:1024 + 64 * hB + 64],
                             w_v_red[:, 64 * hA:64 * hA + 64],
                             w_v_red[:, 64 * hB:64 * hB + 64]], 1)
        wp = np.concatenate([w_proj[64 * hA:64 * hA + 64, :],
                             w_proj[64 * hB:64 * hB + 64, :],
                             w_proj[512 + 64 * hA:512 + 64 * hA + 64, :],
                             w_proj[512 + 64 * hB:512 + 64 * hB + 64, :]], 0)
        tile3 = lambda w: np.ascontiguousarray(
            w.reshape(-1, 128, w.shape[-1]).transpose(1, 0, 2)).astype(bf)
        in_maps.append({
            "xT": np.ascontiguousarray(x[b].T).astype(bf),
            "wq": tile3(wq), "wk": tile3(wk), "wqkr": tile3(wqkr),
            "wv": tile3(wv), "wproj": tile3(wp),
        })
    return in_maps


def kernel(x, w_qkv_full, w_qk_red, w_v_red, w_proj):
    nc = _get_nc()
    in_maps = make_in_maps(x, w_qkv_full, w_qk_red, w_v_red, w_proj)
    r = bass_utils.run_bass_kernel_spmd(nc, in_maps,
                                        core_ids=list(range(N_CORES)),
                                        trace=False)
    outs = [np.asarray(r.results[c]["o"], dtype=np.float32)
            for c in range(N_CORES)]
    y = np.zeros((B, T, C), np.float32)
    for b in range(B):
        y[b] = outs[4 * b] + outs[4 * b + 1] + outs[4 * b + 2] + outs[4 * b + 3]
    return y
